# revision 15
# baseline (speedup 1.0000x reference)
"""CandidateFinder kernel for Trainium2 (8 NeuronCores, SPMD).

Problem: for each query i (per batch), find keys j where
  lsh_match(i,j) = any of 4 LSH hash buckets agree, AND
  trie_match(i,j) = all 12 sign bits of (batch -1) features agree.
Output [B, Sq, 64] int32: if count<=64, ascending candidate indices
right-aligned with -1 padding; if count>64, ascending top-64 by dot-sim.

Device strategy (v2): one matmul + one constant-threshold pass per pair.
  - The gaussian inputs only populate a handful of the 32 LSH buckets per
    hash (~30 distinct buckets total across the 4 hashes). Host remaps each
    hash's occurring bucket values to a compact one-hot (30 dims) and
    appends the 12 trie sign dims (keys sgn in {-1,+1}, queries 2*sgn in
    {-2,+2}), zero-padded to K=64:
      s = lshdot + 2*signdot,  match <=> s >= 24.5
    (signdot=12 gives s = 24+lshdot, so s>=25 iff any hash agrees;
     signdot<=10 gives s <= 20+4 = 24.)  All values exact in fp8/f32.
  - Batch 0's encoding lives in partitions 0..63, batch 1's in 64..127, so
    the two per-key-tile matmuls occupy disjoint PE row groups and run
    concurrently (row-tiled K=64).
  - The threshold pass splits each [128, 1024] PSUM tile between DVE
    (tensor_scalar is_ge -> fp8 0/1) and ACT (Relu(s-24.5) -> fp8,
    nonzero iff match), the two fastest PSUM-reading engines.
  - Mask bytes ship to HBM; host decodes candidate indices exactly,
    right-aligns with -1 padding, and handles the (astronomically rare)
    count>64 top-k branch with an exact host fallback.
"""

import numpy as np
from ml_dtypes import float8_e4m3

import concourse.bacc as bacc
import concourse.tile as tile
from concourse import mybir
from concourse.bass_utils import run_bass_kernel_spmd

B, S, D = 2, 4096, 12
H, BUCKETS, BW = 4, 32, 4.0
KMAX = 64
NCORES = 8
QPC = S // NCORES          # 512 query indices per core (x2 batches)
NKT = S // 128             # 32 key tiles
KDIM = 64                  # padded contraction dims per batch (<=64 required)
SPLIT = 1012               # per 2-kt tile: DVE cols [0:SPLIT), ACT [SPLIT:2048)
THRESH = 24.5

TRACE = False              # set True (module flag) to capture an NTFF trace
LAST_RESULTS = None

_nc_cache = None


def _build():
    global _nc_cache
    if _nc_cache is not None:
        return _nc_cache
    nc = bacc.Bacc()
    f8 = mybir.dt.float8e4
    f32 = mybir.dt.float32

    ft_d = nc.dram_tensor("ft", [128, QPC], f8, kind="ExternalInput")
    gt_d = nc.dram_tensor("gt", [128, S], f8, kind="ExternalInput")
    # per 2-key-tile chunk: DVE writes cols [0:SPLIT), ACT [SPLIT:2048)
    outv_d = nc.dram_tensor("outv", [NKT // 2, 128, SPLIT], f8,
                            kind="ExternalOutput")
    outa_d = nc.dram_tensor("outa", [NKT // 2, 128, 2048 - SPLIT], f8,
                            kind="ExternalOutput")

    with tile.TileContext(nc) as tc:
        with (
            tc.tile_pool(name="keys", bufs=1) as pool_k,
            tc.tile_pool(name="qrs", bufs=1) as pool_q,
            tc.tile_pool(name="mskv", bufs=6) as pool_mv,
            tc.tile_pool(name="mska", bufs=6) as pool_ma,
            tc.tile_pool(name="ps_a", bufs=2, space="PSUM") as pool_pa,
        ):
            bias_t = pool_q.tile([128, 1], f32, tag="bias")
            nc.gpsimd.memset(bias_t[:], -THRESH)
            # inputs ride the scalar HWDGE ring (sync ring is for mask
            # stores); batch-0 halves first so the first matmuls' DMA
            # completion receipt (~2us) starts ticking as early as possible
            f_t = pool_q.tile([128, QPC], f8, tag="ft")
            g_ts = [pool_k.tile([128, 1024], f8, tag=f"gt{c}", name=f"gt{c}")
                    for c in range(4)]
            nc.scalar.dma_start(out=f_t[0:KDIM], in_=ft_d[0:KDIM])
            nc.scalar.dma_start(out=g_ts[0][0:KDIM], in_=gt_d[0:KDIM, 0:1024])
            nc.scalar.dma_start(out=f_t[KDIM:128], in_=ft_d[KDIM:128])
            nc.scalar.dma_start(out=g_ts[0][KDIM:128],
                                in_=gt_d[KDIM:128, 0:1024])
            for c in range(1, 4):
                nc.scalar.dma_start(
                    out=g_ts[c][:], in_=gt_d[:, c * 1024:(c + 1) * 1024])

            for g in range(NKT // 2):           # 2 key tiles per iteration
                mv = pool_mv.tile([128, SPLIT], f8, tag="mskv", name=f"mv_{g}")
                ma = pool_ma.tile([128, 2048 - SPLIT], f8, tag="mska",
                                  name=f"ma_{g}")
                psA = pool_pa.tile([128, 4 * QPC], f32)
                for j in range(2):
                    kt = 2 * g + j
                    for b in range(2):
                        nc.tensor.matmul(
                            psA[:, (2 * j + b) * QPC:(2 * j + b + 1) * QPC],
                            lhsT=g_ts[kt // 8][b * KDIM:(b + 1) * KDIM,
                                               (kt % 8) * 128:(kt % 8 + 1) * 128],
                            rhs=f_t[b * KDIM:(b + 1) * KDIM, :],
                            start=True, stop=True,
                        )
                nc.vector.tensor_scalar(
                    mv[:],
                    psA[:, 0:SPLIT],
                    THRESH, None,
                    mybir.AluOpType.is_ge,
                )
                nc.scalar.activation(
                    ma[:],
                    psA[:, SPLIT:2048],
                    mybir.ActivationFunctionType.Relu,
                    bias=bias_t[:], scale=1.0,
                )
                nc.sync.dma_start(out=outv_d[g], in_=mv[:])
                nc.scalar.dma_start(out=outa_d[g], in_=ma[:])

    nc.compile()  # wait legalization + reg alloc (bass2jax does not finalize)
    _nc_cache = nc
    return nc


def _hashes(x, proj):
    # mirror: floor((x @ lsh_proj) / BW).astype(int32) % BUCKETS
    d = x.astype(np.float32) @ proj.astype(np.float32)
    return np.floor(d / BW).astype(np.int32) % BUCKETS


def _prep(q, k, proj):
    qh = _hashes(q, proj)                       # [B,S,4]
    kh = _hashes(k, proj)
    sq = np.where(q[-1] > 0, np.float32(1.0), np.float32(-1.0))   # [S,12]
    sk = np.where(k[-1] > 0, np.float32(1.0), np.float32(-1.0))

    # Compact per-hash bucket remap: only values that actually occur get a
    # one-hot slot.  offs[h] = base row of hash h's block.
    luts, offs, base = [], [], 0
    for h in range(H):
        vals = np.unique(np.concatenate(
            [qh[:, :, h].ravel(), kh[:, :, h].ravel()]))
        lut = np.full(BUCKETS, -1, np.int32)
        lut[vals] = np.arange(len(vals), dtype=np.int32)
        luts.append(lut)
        offs.append(base)
        base += len(vals)
    n_oh = base
    kdim = n_oh + D                             # used contraction dims
    if kdim > KDIM:
        return qh, kh, sq, sk, None, None, kdim

    # encodings: [128, n] fp8 with batch b in partition rows b*KDIM..
    def encode(hsh, sgn, sign_scale):
        n = hsh.shape[1]
        enc = np.zeros((128, n), np.float32)
        for b in range(B):
            r0 = b * KDIM
            for h in range(H):
                slot = luts[h][hsh[b, :, h]] + offs[h]   # [n], all >= 0
                enc[r0 + slot, np.arange(n)] = 1.0
            enc[r0 + n_oh:r0 + n_oh + D, :] = sign_scale * sgn.T
        return enc.astype(float8_e4m3)

    ft = encode(qh, sq, 2.0)                    # [128, S] queries
    gt = encode(kh, sk, 1.0)                    # [128, S] keys
    return qh, kh, sq, sk, ft, gt, kdim


def _mask_row(b, i, qh, kh, sq, sk):
    lsh = (qh[b, i][None, :] == kh[b]).any(-1)                  # [S]
    trie = (sq[i][None, :] == sk).all(-1)                       # [S]
    return lsh & trie


def _topk_row(q, k, b, i, maskrow):
    sims = q[b, i].astype(np.float32) @ k[b].astype(np.float32).T
    vals = np.where(maskrow, sims, -np.inf)
    top = np.argsort(-vals, kind="stable")[:KMAX]               # jax top_k tiebreak
    return np.sort(top).astype(np.int32)


def _pack(match, q, k, qh, kh, sq, sk):
    """bool match grid [B, Sq, Sk] -> output [B, S, KMAX] int32."""
    cb, cq, ci = np.nonzero(match)
    rowid = cb.astype(np.int64) * S + cq
    counts = np.bincount(rowid, minlength=B * S)
    starts = np.concatenate(([0], np.cumsum(counts)))[:-1]
    ranks = np.arange(len(ci)) - starts[rowid]

    out = np.full((B * S, KMAX), -1, np.int32)
    cnt_row = counts[rowid]
    ok = cnt_row <= KMAX
    out[rowid[ok], (KMAX - cnt_row + ranks)[ok]] = ci[ok]

    # exact host fallback for count > KMAX rows (never happens in practice)
    for r in np.nonzero(counts > KMAX)[0]:
        b, i = divmod(int(r), S)
        mrow = _mask_row(b, i, qh, kh, sq, sk)
        out[r] = _topk_row(q, k, b, i, mrow)

    return out.reshape(B, S, KMAX)


def _ensure_ntff_hook():
    """The container's antenv stub lacks axon_hooks; synthesize it from the
    boot module's ctypes NTFF helper so trace=True can capture HW timings."""
    import sys
    import types
    try:
        from antenv.axon_hooks import get_axon_ntff_profile_hook  # noqa: F401
        return
    except ImportError:
        pass
    from trn_agent_boot.trn_boot import _ntff_profile_via_ctypes
    hook = _ntff_profile_via_ctypes("/opt/axon/libaxon_pjrt.so")
    mod = types.ModuleType("antenv.axon_hooks")
    state = {"hook": hook}
    mod.get_axon_ntff_profile_hook = lambda: state["hook"]
    mod.set_axon_ntff_profile_hook = lambda h: state.update(hook=h)
    import antenv
    antenv.axon_hooks = mod
    sys.modules["antenv.axon_hooks"] = mod


def kernel(**inputs):
    global LAST_RESULTS
    q = np.asarray(inputs["query_features_up"], np.float32)
    k = np.asarray(inputs["key_features_up"], np.float32)
    proj = np.asarray(inputs["lsh_proj"], np.float32)

    qh, kh, sq, sk, ft, gt, kdim = _prep(q, k, proj)
    if ft is None:
        # pathological bucket spread (never with gaussian data): exact host path
        lsh = (qh[:, :, None, :] == kh[:, None, :, :]).any(-1)
        trie = (sq[:, None, :] == sk[None, :, :]).all(-1)
        return _pack(lsh & trie[None], q, k, qh, kh, sq, sk)

    nc = _build()
    in_maps = []
    for c in range(NCORES):
        qoff = c * QPC
        in_maps.append({
            "ft": np.ascontiguousarray(ft[:, qoff:qoff + QPC]),
            "gt": gt,
        })
    if TRACE:
        _ensure_ntff_hook()
    res = run_bass_kernel_spmd(
        nc, in_maps, core_ids=list(range(NCORES)), trace=TRACE
    )
    LAST_RESULTS = res

    # raw mask bytes -> bool match grid [B, Sq, Sk]
    match = np.empty((B, S, S), np.bool_)
    for c in range(NCORES):
        raw = np.concatenate(
            [res.results[c]["outv"].view(np.uint8),
             res.results[c]["outa"].view(np.uint8)], axis=-1)  # [16, 128, 2048]
        # col = (j * 2 + b) * 512 + n ; key = (g*2 + j)*128 + p
        m = (raw & 0x7F).reshape(16, 128, 2, 2, QPC) != 0   # [g, p, j, b, n]
        match[:, c * QPC:(c + 1) * QPC, :] = (
            m.transpose(3, 4, 0, 2, 1).reshape(2, QPC, S))
    return _pack(match, q, k, qh, kh, sq, sk)
